# revision 14
# baseline (speedup 1.0000x reference)
"""Child-Sum Tree-LSTM cell on 8 Trainium2 NeuronCores.

Strategy (data-parallel over the node axis, per the sharding hint):
  - Shard N=8192 nodes across 8 cores (1024 nodes/core); replicate the 8
    W/U weight matrices. No cross-core communication.
  - All device compute runs in a TRANSPOSED layout [H, nodes]: the host
    pre-transposes x/child_h/child_c during sharding so every matmul has
    its contraction dim on SBUF partitions with no on-device transposes,
    and the per-H bias vectors become per-partition activation biases.
  - Gate matmuls use float32r (full fp32 storage, 1 cycle/row at moving
    free-dim >= 256). The per-child forget-gate term x@Wf is computed
    once per node chunk and replicated across the 8 children inside the
    PSUM accumulation with a single identity matmul (stride-0 rhs).
  - The child tensors dominate HBM traffic (32 of 46 MiB/core); they are
    host-cast to bf16 (halving that traffic) while the x/W/U gate paths
    stay fp32r. f32 accumulation everywhere.
"""

import numpy as np

N, K, DIN, H = 8192, 8, 512, 512
NCORES = 8
NL = N // NCORES          # 1024 nodes per core
P = 128
NCH = 512                 # node chunk (matmul moving free dim)
NNCH = NL // NCH          # 4 node chunks
CCH = 512                 # (node, k) column chunk for the f einsum
NPC = CCH // K            # 64 nodes per column chunk
JCH = NCH * K // CCH      # 4 column chunks per node chunk

GATES = "ifou"            # bias column order: [g*4 + hc]

BF16_CHILD = True         # child_h/child_c/Uf in bf16 (halves child DMA)
BF16_ALL = True           # x/W/U/hsum/xwf in bf16 too (FWL weight loads)

_CACHE = {}


def _build_bass():
    import concourse.mybir as mybir
    import concourse.tile as tile
    from concourse import bacc
    from contextlib import ExitStack

    f32 = mybir.dt.float32
    f32r = mybir.dt.float32r
    bf16 = mybir.dt.bfloat16
    ch_dt = bf16 if BF16_CHILD else f32r
    cc_dt = bf16 if BF16_CHILD else f32
    ft_dt = bf16 if BF16_CHILD else f32
    mm_dt = bf16 if BF16_ALL else f32r
    AF = mybir.ActivationFunctionType
    AX = mybir.AxisListType

    # fp32r matmul operands must come from fp32r-typed producer chains
    # (walrus birverifier); all matmul-feeding tensors are declared f32r.
    nc = bacc.Bacc(None, target_bir_lowering=False)

    xT = nc.declare_dram_parameter("xT", [DIN, NL], mm_dt, isOutput=False)
    chT = nc.declare_dram_parameter("chT", [H, NL * K], ch_dt, isOutput=False)
    ccT = nc.declare_dram_parameter("ccT", [H, NL * K], cc_dt, isOutput=False)
    Wd = {g: nc.declare_dram_parameter("W" + g, [DIN, H], mm_dt, isOutput=False)
          for g in GATES}
    Ud = {g: nc.declare_dram_parameter("U" + g, [H, H],
                                       ch_dt if g == "f" else mm_dt,
                                       isOutput=False)
          for g in GATES}
    bias_d = nc.declare_dram_parameter("bias", [P, 16], f32, isOutput=False)
    ident_d = nc.declare_dram_parameter("ident", [P, P], mm_dt, isOutput=False)
    hc_out = nc.declare_dram_parameter("hc_out", [2, H, NL], f32, isOutput=True)

    with tile.TileContext(nc) as tc, ExitStack() as ctx:
        wpool = ctx.enter_context(tc.tile_pool(name="weights", bufs=1))
        io = ctx.enter_context(tc.tile_pool(name="io", bufs=3))
        act = ctx.enter_context(tc.tile_pool(name="acts", bufs=2))
        psA = ctx.enter_context(tc.tile_pool(name="psA", bufs=4, space="PSUM"))
        psB = ctx.enter_context(tc.tile_pool(name="psB", bufs=4, space="PSUM"))

        # Resident weights: [128, dc, h] with dc = contraction-dim chunk.
        # Only the f-gate weights (needed by the pipeline that starts
        # immediately) load upfront; i/o/u weights are deferred into the
        # first phase-B window so TensorE isn't stalled behind their DMA.
        wsb = {}
        for g in GATES:
            wdt = ch_dt if g == "f" else mm_dt
            wsb["W" + g] = wpool.tile([P, 4, H], mm_dt, tag=f"W{g}", name=f"W{g}")
            wsb["U" + g] = wpool.tile([P, 4, H], wdt, tag=f"U{g}", name=f"U{g}")

        def load_weights(gates):
            for g in gates:
                nc.scalar.dma_start(
                    out=wsb["W" + g],
                    in_=Wd[g].rearrange("(dc p) h -> p dc h", p=P))
                nc.scalar.dma_start(
                    out=wsb["U" + g],
                    in_=Ud[g].rearrange("(dc p) h -> p dc h", p=P))

        load_weights("f")
        bias_sb = wpool.tile([P, 16], f32, tag="bias")
        ident = wpool.tile([P, P], mm_dt, tag="ident")

        def bias_ap(g, hc):
            col = 4 * GATES.index(g) + hc
            return bias_sb[:, col:col + 1]

        for t in range(NNCH):
            n0 = t * NCH
            xt = act.tile([P, 4, NCH], mm_dt, tag="xt")
            nc.scalar.dma_start(
                out=xt,
                in_=xT[:, n0:n0 + NCH].rearrange("(dc p) n -> p dc n", p=P))
            if t == 0:
                # bias/ident and the i/o/u weights ride the scalar ring
                # behind Wf/Uf/xt so the f-gate pipeline starts immediately;
                # they all land before phase C of the first chunk needs them.
                nc.scalar.dma_start(out=bias_sb, in_=bias_d[:, :])
                nc.scalar.dma_start(out=ident, in_=ident_d[:, :])
                load_weights("iou")

            # Phase A: xwf[h, n] = x @ Wf + (bWf + bUf), kept for the f gate.
            xwf = act.tile([P, 4, NCH], mm_dt, tag="xwf")
            for hc in range(4):
                ps = psB.tile([P, NCH], f32, tag="ps_small")
                for dc in range(4):
                    nc.tensor.matmul(
                        ps,
                        wsb["Wf"][:, dc, hc * P:(hc + 1) * P],
                        xt[:, dc, :],
                        start=(dc == 0), stop=(dc == 3))
                nc.scalar.activation(xwf[:, hc, :], ps, AF.Identity,
                                     bias=bias_ap("f", hc))

            hsum = act.tile([P, 4, NCH], mm_dt, tag="hsum")
            csum = act.tile([P, 4, NCH], f32, tag="csum")

            # Phase B: children — h_sum, f = sigmoid(Uf@h_k + xwf), f*c sum.
            for j in range(JCH):
                c0 = n0 * K + j * CCH
                ch_t = io.tile([P, 4, CCH], ch_dt, tag="ch")
                nc.sync.dma_start(
                    out=ch_t,
                    in_=chT[:, c0:c0 + CCH].rearrange("(hc p) c -> p hc c", p=P))
                cc_t = io.tile([P, 4, CCH], cc_dt, tag="cc")
                nc.sync.dma_start(
                    out=cc_t,
                    in_=ccT[:, c0:c0 + CCH].rearrange("(hc p) c -> p hc c", p=P))

                with nc.allow_low_precision("f32r feed for U@h_sum matmuls"):
                    nc.vector.reduce_sum(
                        out=hsum[:, :, j * NPC:(j + 1) * NPC],
                        in_=ch_t.rearrange("p hc (n k) -> p hc n k", k=K),
                        axis=AX.X)

                ft = act.tile([P, 4, CCH], ft_dt, tag="ft")
                for hc in range(4):
                    ps = psA.tile([P, CCH], f32, tag="ps_big")
                    for dc in range(4):
                        nc.tensor.matmul(
                            ps,
                            wsb["Uf"][:, dc, hc * P:(hc + 1) * P],
                            ch_t[:, dc, :],
                            start=(dc == 0), stop=False)
                    # += xwf replicated across the 8 children (stride-0 rhs)
                    xrep = (xwf[:, hc, j * NPC:(j + 1) * NPC]
                            .unsqueeze(-1).broadcast_to([P, NPC, K]))
                    nc.tensor.matmul(
                        ps, ident, xrep,
                        start=False, stop=True)
                    nc.scalar.activation(ft[:, hc, :], ps, AF.Sigmoid)

                nc.vector.tensor_mul(ft, ft, cc_t)
                nc.vector.reduce_sum(
                    out=csum[:, :, j * NPC:(j + 1) * NPC],
                    in_=ft.rearrange("p hc (n k) -> p hc n k", k=K),
                    axis=AX.X)


            # Phase C: i/o/u gates, then c and h.
            git = act.tile([P, 4, NCH], f32, tag="git")
            got = act.tile([P, 4, NCH], f32, tag="got")
            gut = act.tile([P, 4, NCH], f32, tag="gut")
            for g, dest in (("i", git), ("o", got), ("u", gut)):
                for hc in range(4):
                    ps = psB.tile([P, NCH], f32, tag="ps_small")
                    for dc in range(4):
                        nc.tensor.matmul(
                            ps,
                            wsb["W" + g][:, dc, hc * P:(hc + 1) * P],
                            xt[:, dc, :],
                            start=(dc == 0), stop=False)
                    for dc in range(4):
                        nc.tensor.matmul(
                            ps,
                            wsb["U" + g][:, dc, hc * P:(hc + 1) * P],
                            hsum[:, dc, :],
                            start=False, stop=(dc == 3))
                    nc.scalar.activation(
                        dest[:, hc, :], ps,
                        AF.Tanh if g == "u" else AF.Sigmoid,
                        bias=bias_ap(g, hc))

            nc.vector.tensor_mul(git, git, gut)       # i*u
            nc.vector.tensor_add(git, git, csum)      # c = i*u + sum_k f*c_k
            nc.scalar.dma_start(
                out=hc_out[1][:, n0:n0 + NCH].rearrange("(hc p) n -> p hc n", p=P),
                in_=git)
            tct = act.tile([P, 4, NCH], f32, tag="tct")
            nc.scalar.activation(tct, git, AF.Tanh)
            nc.vector.tensor_mul(got, got, tct)       # h = o * tanh(c)
            nc.scalar.dma_start(
                out=hc_out[0][:, n0:n0 + NCH].rearrange("(hc p) n -> p hc n", p=P),
                in_=got)

    nc.finalize()
    return nc


def get_nc():
    if "nc" not in _CACHE:
        _CACHE["nc"] = _build_bass()
    return _CACHE["nc"]


def make_in_maps(inputs):
    import ml_dtypes
    f = np.float32
    cdt = ml_dtypes.bfloat16 if BF16_CHILD else f
    mdt = ml_dtypes.bfloat16 if BF16_ALL else f
    x = np.asarray(inputs["x"], f).reshape(NCORES, NL, DIN)
    ch = np.asarray(inputs["child_h"], f).reshape(NCORES, NL, K, H)
    cc = np.asarray(inputs["child_c"], f).reshape(NCORES, NL, K, H)

    bias_cols = []
    for g in GATES:
        bg = (np.asarray(inputs["bW" + g], f)
              + np.asarray(inputs["bU" + g], f))          # [H]
        bias_cols.append(np.ascontiguousarray(bg.reshape(4, P).T))
    bias_packed = np.ascontiguousarray(np.concatenate(bias_cols, axis=1))

    weights = {}
    for g in GATES:
        w = np.ascontiguousarray(np.asarray(inputs["W" + g], f))
        weights["W" + g] = w.astype(mdt)
        u = np.ascontiguousarray(np.asarray(inputs["U" + g], f))
        weights["U" + g] = u.astype(cdt) if g == "f" else u.astype(mdt)

    ident = np.eye(P, dtype=mdt)
    in_maps = []
    for c in range(NCORES):
        m = {
            "ident": ident,
            "xT": np.ascontiguousarray(x[c].T).astype(mdt),
            "chT": np.ascontiguousarray(
                ch[c].transpose(2, 0, 1).reshape(H, NL * K)).astype(cdt),
            "ccT": np.ascontiguousarray(
                cc[c].transpose(2, 0, 1).reshape(H, NL * K)).astype(cdt),
            "bias": bias_packed,
        }
        m.update(weights)
        in_maps.append(m)
    return in_maps


def assemble_output(results):
    h = np.empty((N, H), np.float32)
    c = np.empty((N, H), np.float32)
    for ci in range(NCORES):
        hc = results[ci]["hc_out"]          # [2, H, NL]
        h[ci * NL:(ci + 1) * NL] = hc[0].T
        c[ci * NL:(ci + 1) * NL] = hc[1].T
    return np.stack([h, c], axis=0)


def run(inputs, trace=False):
    from concourse.bass_utils import run_bass_kernel_spmd
    nc = get_nc()
    in_maps = make_in_maps(inputs)
    res = run_bass_kernel_spmd(nc, in_maps, list(range(NCORES)), trace=trace)
    return assemble_output(res.results), res


def kernel(**inputs):
    out, _ = run(inputs, trace=False)
    return out


# revision 16
# speedup vs baseline: 1.0413x; 1.0413x over previous
"""Child-Sum Tree-LSTM cell on 8 Trainium2 NeuronCores.

Strategy (data-parallel over the node axis, per the sharding hint):
  - Shard N=8192 nodes across 8 cores (1024 nodes/core); replicate the 8
    W/U weight matrices. No cross-core communication.
  - All device compute runs in a TRANSPOSED layout [H, nodes]: the host
    pre-transposes x/child_h/child_c during sharding so every matmul has
    its contraction dim on SBUF partitions with no on-device transposes,
    and the per-H bias vectors become per-partition activation biases.
  - Gate matmuls use float32r (full fp32 storage, 1 cycle/row at moving
    free-dim >= 256). The per-child forget-gate term x@Wf is computed
    once per node chunk and replicated across the 8 children inside the
    PSUM accumulation with a single identity matmul (stride-0 rhs).
  - The child tensors dominate HBM traffic (32 of 46 MiB/core); they are
    host-cast to bf16 (halving that traffic) while the x/W/U gate paths
    stay fp32r. f32 accumulation everywhere.
"""

import numpy as np

N, K, DIN, H = 8192, 8, 512, 512
NCORES = 8
NL = N // NCORES          # 1024 nodes per core
P = 128
NCH = 256                 # node chunk (matmul moving free dim)
NNCH = NL // NCH          # 4 node chunks
CCH = 512                 # (node, k) column chunk for the f einsum
NPC = CCH // K            # 64 nodes per column chunk
JCH = NCH * K // CCH      # 4 column chunks per node chunk

GATES = "ifou"            # bias column order: [g*4 + hc]

BF16_CHILD = True         # child_h/child_c/Uf in bf16 (halves child DMA)
BF16_ALL = True           # x/W/U/hsum/xwf in bf16 too (FWL weight loads)

_CACHE = {}


def _build_bass():
    import concourse.mybir as mybir
    import concourse.tile as tile
    from concourse import bacc
    from contextlib import ExitStack

    f32 = mybir.dt.float32
    f32r = mybir.dt.float32r
    bf16 = mybir.dt.bfloat16
    ch_dt = bf16 if BF16_CHILD else f32r
    cc_dt = bf16 if BF16_CHILD else f32
    ft_dt = bf16 if BF16_CHILD else f32
    mm_dt = bf16 if BF16_ALL else f32r
    AF = mybir.ActivationFunctionType
    AX = mybir.AxisListType

    # fp32r matmul operands must come from fp32r-typed producer chains
    # (walrus birverifier); all matmul-feeding tensors are declared f32r.
    nc = bacc.Bacc(None, target_bir_lowering=False)

    xT = nc.declare_dram_parameter("xT", [DIN, NL], mm_dt, isOutput=False)
    chT = nc.declare_dram_parameter("chT", [H, NL * K], ch_dt, isOutput=False)
    ccT = nc.declare_dram_parameter("ccT", [H, NL * K], cc_dt, isOutput=False)
    Wd = {g: nc.declare_dram_parameter("W" + g, [DIN, H], mm_dt, isOutput=False)
          for g in GATES}
    Ud = {g: nc.declare_dram_parameter("U" + g, [H, H],
                                       ch_dt if g == "f" else mm_dt,
                                       isOutput=False)
          for g in GATES}
    bias_d = nc.declare_dram_parameter("bias", [P, 16], f32, isOutput=False)
    ident_d = nc.declare_dram_parameter("ident", [P, P], mm_dt, isOutput=False)
    hc_out = nc.declare_dram_parameter("hc_out", [2, H, NL], f32, isOutput=True)

    with tile.TileContext(nc) as tc, ExitStack() as ctx:
        wpool = ctx.enter_context(tc.tile_pool(name="weights", bufs=1))
        io = ctx.enter_context(tc.tile_pool(name="io", bufs=3))
        act = ctx.enter_context(tc.tile_pool(name="acts", bufs=2))
        psA = ctx.enter_context(tc.tile_pool(name="psA", bufs=4, space="PSUM"))
        psB = ctx.enter_context(tc.tile_pool(name="psB", bufs=4, space="PSUM"))

        # Resident weights: [128, dc, h] with dc = contraction-dim chunk.
        # Only the f-gate weights (needed by the pipeline that starts
        # immediately) load upfront; i/o/u weights are deferred into the
        # first phase-B window so TensorE isn't stalled behind their DMA.
        wsb = {}
        for g in GATES:
            wdt = ch_dt if g == "f" else mm_dt
            wsb["W" + g] = wpool.tile([P, 4, H], mm_dt, tag=f"W{g}", name=f"W{g}")
            wsb["U" + g] = wpool.tile([P, 4, H], wdt, tag=f"U{g}", name=f"U{g}")

        def load_weights(gates):
            for g in gates:
                nc.scalar.dma_start(
                    out=wsb["W" + g],
                    in_=Wd[g].rearrange("(dc p) h -> p dc h", p=P))
                nc.scalar.dma_start(
                    out=wsb["U" + g],
                    in_=Ud[g].rearrange("(dc p) h -> p dc h", p=P))

        load_weights("f")
        bias_sb = wpool.tile([P, 16], f32, tag="bias")
        ident = wpool.tile([P, P], mm_dt, tag="ident")

        def bias_ap(g, hc):
            col = 4 * GATES.index(g) + hc
            return bias_sb[:, col:col + 1]

        for t in range(NNCH):
            n0 = t * NCH
            xt = act.tile([P, 4, NCH], mm_dt, tag="xt")
            nc.scalar.dma_start(
                out=xt,
                in_=xT[:, n0:n0 + NCH].rearrange("(dc p) n -> p dc n", p=P))
            if t == 0:
                # bias/ident and the i/o/u weights ride the scalar ring
                # behind Wf/Uf/xt so the f-gate pipeline starts immediately;
                # they all land before phase C of the first chunk needs them.
                nc.scalar.dma_start(out=bias_sb, in_=bias_d[:, :])
                nc.scalar.dma_start(out=ident, in_=ident_d[:, :])
                load_weights("iou")

            # Phase A: xwf[h, n] = x @ Wf + (bWf + bUf), kept for the f gate.
            xwf = act.tile([P, 4, NCH], mm_dt, tag="xwf")
            for hc in range(4):
                ps = psB.tile([P, NCH], f32, tag="ps_small")
                for dc in range(4):
                    nc.tensor.matmul(
                        ps,
                        wsb["Wf"][:, dc, hc * P:(hc + 1) * P],
                        xt[:, dc, :],
                        start=(dc == 0), stop=(dc == 3))
                nc.scalar.activation(xwf[:, hc, :], ps, AF.Identity,
                                     bias=bias_ap("f", hc))

            hsum = act.tile([P, 4, NCH], mm_dt, tag="hsum")
            csum = act.tile([P, 4, NCH], f32, tag="csum")

            # Phase B: children — h_sum, f = sigmoid(Uf@h_k + xwf), f*c sum.
            for j in range(JCH):
                c0 = n0 * K + j * CCH
                ch_t = io.tile([P, 4, CCH], ch_dt, tag="ch")
                nc.sync.dma_start(
                    out=ch_t,
                    in_=chT[:, c0:c0 + CCH].rearrange("(hc p) c -> p hc c", p=P))
                cc_t = io.tile([P, 4, CCH], cc_dt, tag="cc")
                nc.sync.dma_start(
                    out=cc_t,
                    in_=ccT[:, c0:c0 + CCH].rearrange("(hc p) c -> p hc c", p=P))

                with nc.allow_low_precision("f32r feed for U@h_sum matmuls"):
                    nc.vector.reduce_sum(
                        out=hsum[:, :, j * NPC:(j + 1) * NPC],
                        in_=ch_t.rearrange("p hc (n k) -> p hc n k", k=K),
                        axis=AX.X)

                ft = act.tile([P, 4, CCH], ft_dt, tag="ft")
                for hc in range(4):
                    ps = psA.tile([P, CCH], f32, tag="ps_big")
                    for dc in range(4):
                        nc.tensor.matmul(
                            ps,
                            wsb["Uf"][:, dc, hc * P:(hc + 1) * P],
                            ch_t[:, dc, :],
                            start=(dc == 0), stop=False)
                    # += xwf replicated across the 8 children (stride-0 rhs)
                    xrep = (xwf[:, hc, j * NPC:(j + 1) * NPC]
                            .unsqueeze(-1).broadcast_to([P, NPC, K]))
                    nc.tensor.matmul(
                        ps, ident, xrep,
                        start=False, stop=True)
                    nc.scalar.activation(ft[:, hc, :], ps, AF.Sigmoid)

                nc.vector.tensor_mul(ft, ft, cc_t)
                nc.vector.reduce_sum(
                    out=csum[:, :, j * NPC:(j + 1) * NPC],
                    in_=ft.rearrange("p hc (n k) -> p hc n k", k=K),
                    axis=AX.X)


            # Phase C: i/o/u gates, then c and h — hc-outer so the
            # per-hc c/h elementwise chain and stores overlap the next
            # hc's gate matmuls instead of serializing at the chunk end.
            git = act.tile([P, 4, NCH], f32, tag="git")
            got = act.tile([P, 4, NCH], f32, tag="got")
            gut = act.tile([P, 4, NCH], f32, tag="gut")
            tct = act.tile([P, 4, NCH], f32, tag="tct")
            c_slab = hc_out[1][:, n0:n0 + NCH].rearrange("(hc p) n -> p hc n", p=P)
            h_slab = hc_out[0][:, n0:n0 + NCH].rearrange("(hc p) n -> p hc n", p=P)
            for hc in range(4):
                for g, dest in (("i", git), ("o", got), ("u", gut)):
                    ps = psB.tile([P, NCH], f32, tag="ps_small")
                    for dc in range(4):
                        nc.tensor.matmul(
                            ps,
                            wsb["W" + g][:, dc, hc * P:(hc + 1) * P],
                            xt[:, dc, :],
                            start=(dc == 0), stop=False)
                    for dc in range(4):
                        nc.tensor.matmul(
                            ps,
                            wsb["U" + g][:, dc, hc * P:(hc + 1) * P],
                            hsum[:, dc, :],
                            start=False, stop=(dc == 3))
                    nc.scalar.activation(
                        dest[:, hc, :], ps,
                        AF.Tanh if g == "u" else AF.Sigmoid,
                        bias=bias_ap(g, hc))
                nc.vector.tensor_mul(git[:, hc, :], git[:, hc, :], gut[:, hc, :])
                nc.vector.tensor_add(git[:, hc, :], git[:, hc, :], csum[:, hc, :])
                nc.scalar.dma_start(out=c_slab[:, hc, :], in_=git[:, hc, :])
                nc.scalar.activation(tct[:, hc, :], git[:, hc, :], AF.Tanh)
                nc.vector.tensor_mul(got[:, hc, :], got[:, hc, :], tct[:, hc, :])
                nc.scalar.dma_start(out=h_slab[:, hc, :], in_=got[:, hc, :])

    nc.finalize()
    return nc


def get_nc():
    if "nc" not in _CACHE:
        _CACHE["nc"] = _build_bass()
    return _CACHE["nc"]


def make_in_maps(inputs):
    import ml_dtypes
    f = np.float32
    cdt = ml_dtypes.bfloat16 if BF16_CHILD else f
    mdt = ml_dtypes.bfloat16 if BF16_ALL else f
    x = np.asarray(inputs["x"], f).reshape(NCORES, NL, DIN)
    ch = np.asarray(inputs["child_h"], f).reshape(NCORES, NL, K, H)
    cc = np.asarray(inputs["child_c"], f).reshape(NCORES, NL, K, H)

    bias_cols = []
    for g in GATES:
        bg = (np.asarray(inputs["bW" + g], f)
              + np.asarray(inputs["bU" + g], f))          # [H]
        bias_cols.append(np.ascontiguousarray(bg.reshape(4, P).T))
    bias_packed = np.ascontiguousarray(np.concatenate(bias_cols, axis=1))

    weights = {}
    for g in GATES:
        w = np.ascontiguousarray(np.asarray(inputs["W" + g], f))
        weights["W" + g] = w.astype(mdt)
        u = np.ascontiguousarray(np.asarray(inputs["U" + g], f))
        weights["U" + g] = u.astype(cdt) if g == "f" else u.astype(mdt)

    ident = np.eye(P, dtype=mdt)
    in_maps = []
    for c in range(NCORES):
        m = {
            "ident": ident,
            "xT": np.ascontiguousarray(x[c].T).astype(mdt),
            "chT": np.ascontiguousarray(
                ch[c].transpose(2, 0, 1).reshape(H, NL * K)).astype(cdt),
            "ccT": np.ascontiguousarray(
                cc[c].transpose(2, 0, 1).reshape(H, NL * K)).astype(cdt),
            "bias": bias_packed,
        }
        m.update(weights)
        in_maps.append(m)
    return in_maps


def assemble_output(results):
    h = np.empty((N, H), np.float32)
    c = np.empty((N, H), np.float32)
    for ci in range(NCORES):
        hc = results[ci]["hc_out"]          # [2, H, NL]
        h[ci * NL:(ci + 1) * NL] = hc[0].T
        c[ci * NL:(ci + 1) * NL] = hc[1].T
    return np.stack([h, c], axis=0)


def run(inputs, trace=False):
    from concourse.bass_utils import run_bass_kernel_spmd
    nc = get_nc()
    in_maps = make_in_maps(inputs)
    res = run_bass_kernel_spmd(nc, in_maps, list(range(NCORES)), trace=trace)
    return assemble_output(res.results), res


def kernel(**inputs):
    out, _ = run(inputs, trace=False)
    return out


# revision 17
# speedup vs baseline: 1.0808x; 1.0380x over previous
"""Child-Sum Tree-LSTM cell on 8 Trainium2 NeuronCores.

Strategy (data-parallel over the node axis, per the sharding hint):
  - Shard N=8192 nodes across 8 cores (1024 nodes/core); replicate the 8
    W/U weight matrices. No cross-core communication.
  - All device compute runs in a TRANSPOSED layout [H, nodes]: the host
    pre-transposes x/child_h/child_c during sharding so every matmul has
    its contraction dim on SBUF partitions with no on-device transposes,
    and the per-H bias vectors become per-partition activation biases.
  - Gate matmuls use float32r (full fp32 storage, 1 cycle/row at moving
    free-dim >= 256). The per-child forget-gate term x@Wf is computed
    once per node chunk and replicated across the 8 children inside the
    PSUM accumulation with a single identity matmul (stride-0 rhs).
  - The child tensors dominate HBM traffic (32 of 46 MiB/core); they are
    host-cast to bf16 (halving that traffic) while the x/W/U gate paths
    stay fp32r. f32 accumulation everywhere.
"""

import numpy as np

N, K, DIN, H = 8192, 8, 512, 512
NCORES = 8
NL = N // NCORES          # 1024 nodes per core
P = 128
NCH = 256                 # node chunk (matmul moving free dim)
NNCH = NL // NCH          # 4 node chunks
CCH = 512                 # (node, k) column chunk for the f einsum
NPC = CCH // K            # 64 nodes per column chunk
JCH = NCH * K // CCH      # 4 column chunks per node chunk

GATES = "ifou"            # bias column order: [g*4 + hc]

BF16_CHILD = True         # child_h/child_c/Uf in bf16 (halves child DMA)
BF16_ALL = True           # x/W/U/hsum/xwf in bf16 too (FWL weight loads)

_CACHE = {}


def _build_bass():
    import concourse.mybir as mybir
    import concourse.tile as tile
    from concourse import bacc
    from contextlib import ExitStack

    f32 = mybir.dt.float32
    f32r = mybir.dt.float32r
    bf16 = mybir.dt.bfloat16
    ch_dt = bf16 if BF16_CHILD else f32r
    cc_dt = bf16 if BF16_CHILD else f32
    ft_dt = bf16 if BF16_CHILD else f32
    mm_dt = bf16 if BF16_ALL else f32r
    AF = mybir.ActivationFunctionType
    AX = mybir.AxisListType

    # fp32r matmul operands must come from fp32r-typed producer chains
    # (walrus birverifier); all matmul-feeding tensors are declared f32r.
    nc = bacc.Bacc(None, target_bir_lowering=False)

    xT = nc.declare_dram_parameter("xT", [DIN, NL], mm_dt, isOutput=False)
    chT = nc.declare_dram_parameter("chT", [H, NL * K], ch_dt, isOutput=False)
    ccT = nc.declare_dram_parameter("ccT", [H, NL * K], cc_dt, isOutput=False)
    Wd = {g: nc.declare_dram_parameter("W" + g, [DIN, H], mm_dt, isOutput=False)
          for g in GATES}
    Ud = {g: nc.declare_dram_parameter("U" + g, [H, H],
                                       ch_dt if g == "f" else mm_dt,
                                       isOutput=False)
          for g in GATES}
    bias_d = nc.declare_dram_parameter("bias", [P, 16], f32, isOutput=False)
    ident_d = nc.declare_dram_parameter("ident", [P, P], mm_dt, isOutput=False)
    hc_out = nc.declare_dram_parameter("hc_out", [2, H, NL], f32, isOutput=True)

    with tile.TileContext(nc) as tc, ExitStack() as ctx:
        wpool = ctx.enter_context(tc.tile_pool(name="weights", bufs=1))
        io = ctx.enter_context(tc.tile_pool(name="io", bufs=4))
        act = ctx.enter_context(tc.tile_pool(name="acts", bufs=2))
        psA = ctx.enter_context(tc.tile_pool(name="psA", bufs=4, space="PSUM"))
        psB = ctx.enter_context(tc.tile_pool(name="psB", bufs=4, space="PSUM"))

        # Resident weights: [128, dc, h] with dc = contraction-dim chunk.
        # Only the f-gate weights (needed by the pipeline that starts
        # immediately) load upfront; i/o/u weights are deferred into the
        # first phase-B window so TensorE isn't stalled behind their DMA.
        wsb = {}
        for g in GATES:
            wdt = ch_dt if g == "f" else mm_dt
            wsb["W" + g] = wpool.tile([P, 4, H], mm_dt, tag=f"W{g}", name=f"W{g}")
            wsb["U" + g] = wpool.tile([P, 4, H], wdt, tag=f"U{g}", name=f"U{g}")

        def load_weights(gates):
            for g in gates:
                nc.scalar.dma_start(
                    out=wsb["W" + g],
                    in_=Wd[g].rearrange("(dc p) h -> p dc h", p=P))
                nc.scalar.dma_start(
                    out=wsb["U" + g],
                    in_=Ud[g].rearrange("(dc p) h -> p dc h", p=P))

        load_weights("f")
        bias_sb = wpool.tile([P, 16], f32, tag="bias")
        ident = wpool.tile([P, P], mm_dt, tag="ident")

        def bias_ap(g, hc):
            col = 4 * GATES.index(g) + hc
            return bias_sb[:, col:col + 1]

        for t in range(NNCH):
            n0 = t * NCH
            xt = act.tile([P, 4, NCH], mm_dt, tag="xt")
            nc.scalar.dma_start(
                out=xt,
                in_=xT[:, n0:n0 + NCH].rearrange("(dc p) n -> p dc n", p=P))
            if t == 0:
                # bias/ident and the i/o/u weights ride the scalar ring
                # behind Wf/Uf/xt so the f-gate pipeline starts immediately;
                # they all land before phase C of the first chunk needs them.
                nc.scalar.dma_start(out=bias_sb, in_=bias_d[:, :])
                nc.scalar.dma_start(out=ident, in_=ident_d[:, :])
                load_weights("iou")

            # Phase A: xwf[h, n] = x @ Wf + (bWf + bUf), kept for the f gate.
            xwf = act.tile([P, 4, NCH], mm_dt, tag="xwf")
            for hc in range(4):
                ps = psB.tile([P, NCH], f32, tag="ps_small")
                for dc in range(4):
                    nc.tensor.matmul(
                        ps,
                        wsb["Wf"][:, dc, hc * P:(hc + 1) * P],
                        xt[:, dc, :],
                        start=(dc == 0), stop=(dc == 3))
                nc.scalar.activation(xwf[:, hc, :], ps, AF.Identity,
                                     bias=bias_ap("f", hc))

            hsum = act.tile([P, 4, NCH], mm_dt, tag="hsum")
            csum = act.tile([P, 4, NCH], f32, tag="csum")

            # Phase B: children — h_sum, f = sigmoid(Uf@h_k + xwf), f*c sum.
            for j in range(JCH):
                c0 = n0 * K + j * CCH
                ch_t = io.tile([P, 4, CCH], ch_dt, tag="ch")
                nc.sync.dma_start(
                    out=ch_t,
                    in_=chT[:, c0:c0 + CCH].rearrange("(hc p) c -> p hc c", p=P))
                cc_t = io.tile([P, 4, CCH], cc_dt, tag="cc")
                nc.sync.dma_start(
                    out=cc_t,
                    in_=ccT[:, c0:c0 + CCH].rearrange("(hc p) c -> p hc c", p=P))

                with nc.allow_low_precision("f32r feed for U@h_sum matmuls"):
                    nc.vector.reduce_sum(
                        out=hsum[:, :, j * NPC:(j + 1) * NPC],
                        in_=ch_t.rearrange("p hc (n k) -> p hc n k", k=K),
                        axis=AX.X)

                ft = act.tile([P, 4, CCH], ft_dt, tag="ft")
                for hc in range(4):
                    ps = psA.tile([P, CCH], f32, tag="ps_big")
                    for dc in range(4):
                        nc.tensor.matmul(
                            ps,
                            wsb["Uf"][:, dc, hc * P:(hc + 1) * P],
                            ch_t[:, dc, :],
                            start=(dc == 0), stop=False)
                    # += xwf replicated across the 8 children (stride-0 rhs)
                    xrep = (xwf[:, hc, j * NPC:(j + 1) * NPC]
                            .unsqueeze(-1).broadcast_to([P, NPC, K]))
                    nc.tensor.matmul(
                        ps, ident, xrep,
                        start=False, stop=True)
                    nc.scalar.activation(ft[:, hc, :], ps, AF.Sigmoid)

                nc.vector.tensor_mul(ft, ft, cc_t)
                nc.vector.reduce_sum(
                    out=csum[:, :, j * NPC:(j + 1) * NPC],
                    in_=ft.rearrange("p hc (n k) -> p hc n k", k=K),
                    axis=AX.X)


            # Phase C: i/o/u gates, then c and h — hc-outer so the
            # per-hc c/h elementwise chain and stores overlap the next
            # hc's gate matmuls instead of serializing at the chunk end.
            git = act.tile([P, 4, NCH], f32, tag="git")
            got = act.tile([P, 4, NCH], f32, tag="got")
            gut = act.tile([P, 4, NCH], f32, tag="gut")
            tct = act.tile([P, 4, NCH], f32, tag="tct")
            c_slab = hc_out[1][:, n0:n0 + NCH].rearrange("(hc p) n -> p hc n", p=P)
            h_slab = hc_out[0][:, n0:n0 + NCH].rearrange("(hc p) n -> p hc n", p=P)
            for hc in range(4):
                for g, dest in (("i", git), ("o", got), ("u", gut)):
                    ps = psB.tile([P, NCH], f32, tag="ps_small")
                    for dc in range(4):
                        nc.tensor.matmul(
                            ps,
                            wsb["W" + g][:, dc, hc * P:(hc + 1) * P],
                            xt[:, dc, :],
                            start=(dc == 0), stop=False)
                    for dc in range(4):
                        nc.tensor.matmul(
                            ps,
                            wsb["U" + g][:, dc, hc * P:(hc + 1) * P],
                            hsum[:, dc, :],
                            start=False, stop=(dc == 3))
                    nc.scalar.activation(
                        dest[:, hc, :], ps,
                        AF.Tanh if g == "u" else AF.Sigmoid,
                        bias=bias_ap(g, hc))
                nc.vector.tensor_mul(git[:, hc, :], git[:, hc, :], gut[:, hc, :])
                nc.vector.tensor_add(git[:, hc, :], git[:, hc, :], csum[:, hc, :])
                nc.sync.dma_start(out=c_slab[:, hc, :], in_=git[:, hc, :])
                nc.scalar.activation(tct[:, hc, :], git[:, hc, :], AF.Tanh)
                nc.vector.tensor_mul(got[:, hc, :], got[:, hc, :], tct[:, hc, :])
                nc.sync.dma_start(out=h_slab[:, hc, :], in_=got[:, hc, :])

    nc.finalize()
    return nc


def get_nc():
    if "nc" not in _CACHE:
        _CACHE["nc"] = _build_bass()
    return _CACHE["nc"]


def make_in_maps(inputs):
    import ml_dtypes
    f = np.float32
    cdt = ml_dtypes.bfloat16 if BF16_CHILD else f
    mdt = ml_dtypes.bfloat16 if BF16_ALL else f
    x = np.asarray(inputs["x"], f).reshape(NCORES, NL, DIN)
    ch = np.asarray(inputs["child_h"], f).reshape(NCORES, NL, K, H)
    cc = np.asarray(inputs["child_c"], f).reshape(NCORES, NL, K, H)

    bias_cols = []
    for g in GATES:
        bg = (np.asarray(inputs["bW" + g], f)
              + np.asarray(inputs["bU" + g], f))          # [H]
        bias_cols.append(np.ascontiguousarray(bg.reshape(4, P).T))
    bias_packed = np.ascontiguousarray(np.concatenate(bias_cols, axis=1))

    weights = {}
    for g in GATES:
        w = np.ascontiguousarray(np.asarray(inputs["W" + g], f))
        weights["W" + g] = w.astype(mdt)
        u = np.ascontiguousarray(np.asarray(inputs["U" + g], f))
        weights["U" + g] = u.astype(cdt) if g == "f" else u.astype(mdt)

    ident = np.eye(P, dtype=mdt)
    in_maps = []
    for c in range(NCORES):
        m = {
            "ident": ident,
            "xT": np.ascontiguousarray(x[c].T).astype(mdt),
            "chT": np.ascontiguousarray(
                ch[c].transpose(2, 0, 1).reshape(H, NL * K)).astype(cdt),
            "ccT": np.ascontiguousarray(
                cc[c].transpose(2, 0, 1).reshape(H, NL * K)).astype(cdt),
            "bias": bias_packed,
        }
        m.update(weights)
        in_maps.append(m)
    return in_maps


def assemble_output(results):
    h = np.empty((N, H), np.float32)
    c = np.empty((N, H), np.float32)
    for ci in range(NCORES):
        hc = results[ci]["hc_out"]          # [2, H, NL]
        h[ci * NL:(ci + 1) * NL] = hc[0].T
        c[ci * NL:(ci + 1) * NL] = hc[1].T
    return np.stack([h, c], axis=0)


def run(inputs, trace=False):
    from concourse.bass_utils import run_bass_kernel_spmd
    nc = get_nc()
    in_maps = make_in_maps(inputs)
    res = run_bass_kernel_spmd(nc, in_maps, list(range(NCORES)), trace=trace)
    return assemble_output(res.results), res


def kernel(**inputs):
    out, _ = run(inputs, trace=False)
    return out
